# revision 43
# baseline (speedup 1.0000x reference)
"""Low_Rank_linear Trainium2 kernel, v14 (152.8us, rel err 1.28e-2).

Per 512-token block (data-parallel over 8 cores, host-permuted inputs,
x pre-transposed feature-major bf16):
    MM-A  hidden.T = (B*wnorm) @ xp.T          k=3840, out 512  bf16
    MM-B  yp[:,:3840] = hid @ A.T + xc @ s1.T  k=768, out 3840  bf16+fp8DR
    MM-C  y2 = (s2p*64) @ xp.T (feature-major) k=4096, out 256  fp8DR

Measured layout (zero mid-kernel tensor gaps): warmups 7.5-14.8,
A0 -> 40.6, C0 -> 47.3 (bridges w2 arrival), B0 -> 80.2, A1 -> 106.3,
C1-mh0 -> 109.8 (bridges the u3-copy chain), B1 -> 142.5,
C1-mh1 -> 146.1, tail ~6.5.  Stream 131.3us vs 128.8 floor.

Design rules (each violated once and measured; do not regress):
  - 106 warm-up matmuls run gaplessly from ~7.5us until the first
    x/w1 chunks land (~14.5us).  HAM locks 8/8 at ~10.8us and sticks;
    idle before the first real matmul resets the ramp.  Start-late is
    cheap (1:1); start-early costs up to 4x in DMA-starve stalls
    (supply jitters +-1.5us), so NDUMMY is sized for the slow case.
  - first w1/x chunks are 2-kt tiles; w1/x pieces of the same kt
    range ride OPPOSITE hw queues so both queues advance the
    consumption frontier together.
  - all late loads (s2, s1, w2, blk1 x) ride the sync queue only: a
    gated DMA-issue instruction blocks its issuing ENGINE, and the
    scalar engine must stay free for the x8 casts -- otherwise the
    compile-time tensor-queue scheduler sees MM-B's DR matmuls
    blocked on xc8 and reorders the whole stream around them.
  - u3 psum->sbuf copies split across vector+scalar (serial 4-copy
    chain otherwise gates MM-B's first LDW); C1-mh0 emitted between
    A1 and B1 to cover what remains.
  - C1-mh1 output NOT split (splitting the final yc mul/DMA measured
    +0.5us in teardown).

Hard limits found (do not retry): gpsimd is a software DMA queue that
steals shared HBM bandwidth (~350-440GB/s total across all queues);
fp8 on the dominant MM-A/MM-B path exceeds the 2e-2 error budget
(each fp8 operand adds ~4%); the A@B' singular spectrum is flat, so
rank truncation / mixed-precision-by-singular-value loses; int8
matmul is unsupported by the toolchain; DoublePixel/DoubleColumn are
uint8-only; DR streams 1 col/cycle (2x via doubled k), so MM-B's
chunk cadence is already within 2.2% of floor.  Fixed costs: 7.5us
framework preamble, ~5us first-DMA latency, ~4us teardown."""

import numpy as np
import ml_dtypes

import concourse.bacc as bacc
import concourse.tile as tile
import concourse.mybir as mybir
from concourse.bass_utils import run_bass_kernel_spmd

N_CORES = 8
TOK = 8192
TPC = TOK // N_CORES  # 1024 tokens per core
N = 4096
RANK = 512
NKEEP = 3840
NCOMP = 256
BLK = 512             # token block (matmul moving N)
TT = 128              # token tile (stationary partition dim)
NBLK = TPC // BLK     # 2
KT_ALL = N // 128     # 32
KT_A = NKEEP // 128   # 30
KT_B = RANK // 128    # 4
NCH = 8
CW = NKEEP // NCH     # 480
XCK = 8               # k-tiles per full x chunk
S1S = 8.0
S2S = 64.0
NDUMMY = 106
HW = NKEEP // 2       # 1920, y half-row width

_BF16 = mybir.dt.bfloat16
_F32 = mybir.dt.float32
_F8 = mybir.dt.float8e4
_DR = mybir.MatmulPerfMode.DoubleRow


def _build_nc():
    nc = bacc.Bacc(None)
    x_d = nc.dram_tensor("x", [NBLK, 4, 128, XCK, BLK], _BF16, kind="ExternalInput")
    w1_d = nc.dram_tensor("w1", [2, 128, 15, RANK], _BF16, kind="ExternalInput")
    w2_d = nc.dram_tensor("w2", [128, KT_B, NKEEP], _BF16, kind="ExternalInput")
    s1_d = nc.dram_tensor("s1", [128, 2, NKEEP], _F8, kind="ExternalInput")
    s2_d = nc.dram_tensor("s2", [128, KT_ALL, NCOMP], _F8, kind="ExternalInput")
    y_d = nc.dram_tensor("y", [TPC, NKEEP], _BF16, kind="ExternalOutput")
    y2_d = nc.dram_tensor("y2", [NCOMP, TPC], _BF16, kind="ExternalOutput")

    with tile.TileContext(nc) as tc:
        with (
            tc.tile_pool(name="w1h2", bufs=2) as w1h2_pool,
            tc.tile_pool(name="w14", bufs=1) as w14_pool,
            tc.tile_pool(name="xh2", bufs=2) as xh2_pool,
            tc.tile_pool(name="w1", bufs=3) as w1_pool,
            tc.tile_pool(name="w2", bufs=4) as w2_pool,
            tc.tile_pool(name="s1", bufs=1) as s1_pool,
            tc.tile_pool(name="s2", bufs=1) as s2_pool,
            tc.tile_pool(name="xt4", bufs=1) as xt4_pool,
            tc.tile_pool(name="xt", bufs=7) as xt_pool,
            tc.tile_pool(name="x8", bufs=1) as x8_pool,
            tc.tile_pool(name="xc8", bufs=2) as xc8_pool,
            tc.tile_pool(name="u3", bufs=2) as u3_pool,
            tc.tile_pool(name="yoa", bufs=2) as yoa_pool,
            tc.tile_pool(name="yob", bufs=2) as yob_pool,
            tc.tile_pool(name="yc", bufs=2) as yc_pool,
            tc.tile_pool(name="wrm", bufs=1) as wrm_pool,
            tc.tile_pool(name="psA", bufs=4, space="PSUM") as psA,
            tc.tile_pool(name="psB", bufs=2, space="PSUM") as psB,
            tc.tile_pool(name="psC", bufs=2, space="PSUM") as psC,
        ):
            # --- tiles ---------------------------------------------------
            # blk0 chunk layout: 2,2,4 k-tiles then three 8kt chunks
            w1h2 = [w1h2_pool.tile([128, 2, RANK], _BF16, name="w1h2t")
                    for _ in range(2)]
            w1h = w14_pool.tile([128, 4, RANK], _BF16, name="w14t")
            w1f = [w1_pool.tile([128, XCK, RANK], _BF16, name="w1sb")
                   for _ in range(3)]
            # (tile, kt_start, nkt) per A-chunk
            w1_chunks = [(w1h2[0], 0, 2), (w1h2[1], 2, 2), (w1h, 4, 4),
                         (w1f[0], 8, 8), (w1f[1], 16, 8), (w1f[2], 24, 6)]
            w2_sb = [w2_pool.tile([128, KT_B, 2 * CW], _BF16, name="w2sb")
                     for c in range(4)]
            s1_sb = s1_pool.tile([128, 2, NKEEP], _F8)
            s2_sb = s2_pool.tile([128, KT_ALL, NCOMP], _F8)
            xh2 = [xh2_pool.tile([128, 2, BLK], _BF16, name="xh2t")
                   for _ in range(2)]
            xt0h = xt4_pool.tile([128, 4, BLK], _BF16, name="xt4t")
            xt_f = [xt_pool.tile([128, XCK, BLK], _BF16, name="xts")
                    for _ in range(7)]
            # per-block x chunk lists: (tile, kt_start, nkt)
            x_chunks = [
                [(xh2[0], 0, 2), (xh2[1], 2, 2), (xt0h, 4, 4),
                 (xt_f[0], 8, 8), (xt_f[1], 16, 8), (xt_f[2], 24, 8)],
                [(xt_f[3], 0, 8), (xt_f[4], 8, 8), (xt_f[5], 16, 8),
                 (xt_f[6], 24, 8)],
            ]
            x8_sb = x8_pool.tile([128, KT_ALL, BLK], _F8)

            # --- warm-up (HAM 8/8 before real MMs) -----------------------
            wrm = wrm_pool.tile([128, 128], _BF16)
            wps = psA.tile([128, 128], _F32, name="psa")
            nc.gpsimd.memset(wrm[:], 0.0)
            for i in range(NDUMMY):
                nc.tensor.matmul(wps[:], wrm[:], wrm[:], start=True, stop=True)

            # --- need-ordered loads (both HW queues ~220GB/s; gpsimd's
            # software queue steals shared bandwidth, so no loads there) --
            # w1/x pieces of the same kt range ride OPPOSITE queues so the
            # two HW queues advance the consumption frontier together
            nc.sync.dma_start(w1h2[0][:], w1_d[0, :, :2])
            nc.scalar.dma_start(xh2[0][:], x_d[0, 0, :, :2, :])
            nc.scalar.dma_start(w1h2[1][:], w1_d[0, :, 2:4])
            nc.sync.dma_start(xh2[1][:], x_d[0, 0, :, 2:4, :])
            nc.sync.dma_start(w1h[:], w1_d[0, :, 4:8])
            nc.scalar.dma_start(xt0h[:], x_d[0, 0, :, 4:, :])
            nc.scalar.dma_start(w1f[0][:, :7, :], w1_d[0, :, 8:])
            nc.sync.dma_start(xt_f[0][:, :4, :], x_d[0, 1, :, :4, :])
            nc.scalar.dma_start(xt_f[0][:, 4:, :], x_d[0, 1, :, 4:, :])
            nc.sync.dma_start(w1f[0][:, 7:8, :], w1_d[1, :, :1])
            nc.sync.dma_start(w1f[1][:], w1_d[1, :, 1:9])
            nc.scalar.dma_start(xt_f[1][:], x_d[0, 2])
            nc.scalar.dma_start(w1f[2][:, :6, :], w1_d[1, :, 9:])
            nc.sync.dma_start(xt_f[2][:], x_d[0, 3])
            # all late loads ride sync: gated DMA-issue instructions block
            # the issuing ENGINE, and scalar must stay free for the x8
            # casts (else the compile-time scheduler defers MM-B's DR
            # matmuls and scrambles the whole tensor queue)
            nc.sync.dma_start(s2_sb[:], s2_d[:])
            nc.sync.dma_start(s1_sb[:], s1_d[:])
            nc.sync.dma_start(w2_sb[0][:], w2_d[:, :, 0:2 * CW])
            nc.sync.dma_start(w2_sb[1][:], w2_d[:, :, 2 * CW:4 * CW])
            nc.sync.dma_start(w2_sb[2][:], w2_d[:, :, 4 * CW:6 * CW])
            nc.sync.dma_start(w2_sb[3][:], w2_d[:, :, 6 * CW:])
            nc.sync.dma_start(xt_f[3][:], x_d[1, 0])
            nc.sync.dma_start(xt_f[4][:], x_d[1, 1])
            nc.sync.dma_start(xt_f[5][:], x_d[1, 2])
            nc.sync.dma_start(xt_f[6][:], x_d[1, 3])

            # --- per-block compute ---------------------------------------
            def mm_a(blk):
                u3 = u3_pool.tile([128, KT_B, BLK], _BF16)
                psa = [psA.tile([128, BLK], _F32, name="psa")
                       for m in range(RANK // 128)]
                for (xt, kt0, nkt) in x_chunks[blk]:
                    for j in range(nkt):
                        kt = kt0 + j
                        if kt >= KT_A:
                            continue
                        ci = (0 if kt < 2 else 1 if kt < 4 else 2 if kt < 8
                              else 3 + (kt - 8) // 8)
                        wt, wkt0, _ = w1_chunks[ci]
                        for m in range(RANK // 128):
                            nc.tensor.matmul(
                                psa[m][:],
                                wt[:, kt - wkt0, m * 128:(m + 1) * 128],
                                xt[:, j, :],
                                start=(kt == 0),
                                stop=(kt == KT_A - 1),
                            )
                    # fp8 cast for MM-C, hidden in MM-A's DMA-paced window
                    nc.scalar.copy(out=x8_sb[:, kt0:kt0 + nkt, :], in_=xt[:])
                # u3 copies split across vector+scalar: the serial 4-copy
                # chain (~2.6us on vector alone) gates MM-B's first LDW
                nc.vector.tensor_copy(out=u3[:, 0, :], in_=psa[0][:])
                nc.scalar.copy(out=u3[:, 2, :], in_=psa[2][:])
                nc.vector.tensor_copy(out=u3[:, 1, :], in_=psa[1][:])
                nc.scalar.copy(out=u3[:, 3, :], in_=psa[3][:])
                xc8 = xc8_pool.tile([128, 2, BLK], _F8)
                nc.scalar.mul(xc8[:], x8_sb[:, 30:32, :], 1.0 / S1S)
                return u3, xc8

            def mm_b(blk, u3, xc8, last=False):
                t0 = blk * BLK
                for mt in range(BLK // TT):
                    yoa = yoa_pool.tile([128, HW], _BF16)
                    yob = yob_pool.tile([128, HW], _BF16)
                    for n in range(NCH):
                        ps = psB.tile([128, CW], _F32)
                        for kt in range(KT_B):
                            nc.tensor.matmul(
                                ps[:],
                                u3[:, kt, mt * TT:(mt + 1) * TT],
                                w2_sb[n // 2][:, kt,
                                              (n % 2) * CW:(n % 2 + 1) * CW],
                                start=(kt == 0),
                                stop=False,
                            )
                        nc.tensor.matmul(
                            ps[:],
                            xc8[:, :, mt * TT:(mt + 1) * TT],
                            s1_sb[:, :, n * CW:(n + 1) * CW],
                            start=False,
                            stop=True,
                            perf_mode=_DR,
                        )
                        dst = yoa if n < 4 else yob
                        nc.vector.tensor_copy(
                            out=dst[:, (n % 4) * CW:(n % 4 + 1) * CW], in_=ps[:]
                        )
                        if n == 3:
                            nc.scalar.dma_start(
                                y_d[t0 + mt * TT:t0 + (mt + 1) * TT, :HW],
                                yoa[:],
                            )
                    r0, r1 = t0 + mt * TT, t0 + (mt + 1) * TT
                    if last and mt == BLK // TT - 1:
                        # the kernel's final row writes gate teardown:
                        # split across both HW queues so they land ~1us
                        # earlier than one serial 480KB transfer
                        nc.scalar.dma_start(y_d[r0:r1, HW:HW + 960],
                                            yob[:, :960])
                        nc.sync.dma_start(y_d[r0:r1, HW + 960:],
                                          yob[:, 960:])
                    else:
                        nc.scalar.dma_start(y_d[r0:r1, HW:], yob[:])

            def mm_c(blk, mh_list=None, out_sync=False):
                t0 = blk * BLK
                for mh in (mh_list or range(NCOMP // 128)):
                    m0, m1 = mh * 128, (mh + 1) * 128
                    ps = psC.tile([128, BLK], _F32)
                    for k2 in range(KT_ALL // 2):
                        nc.tensor.matmul(
                            ps[:],
                            s2_sb[:, 2 * k2:2 * k2 + 2, m0:m1],
                            x8_sb[:, 2 * k2:2 * k2 + 2, :],
                            start=(k2 == 0),
                            stop=(k2 == KT_ALL // 2 - 1),
                            perf_mode=_DR,
                        )
                    yc = yc_pool.tile([128, BLK], _BF16)
                    nc.scalar.mul(yc[:], ps[:], 1.0 / S2S)
                    if out_sync:
                        # final y2 write gates teardown: one mul, then the
                        # write split across both idle HW queues (gpsimd's
                        # software queue needs ~1.3us for the same bytes)
                        nc.sync.dma_start(y2_d[m0:m1, t0:t0 + 256],
                                          yc[:, :256])
                        nc.scalar.dma_start(y2_d[m0:m1, t0 + 256:t0 + BLK],
                                            yc[:, 256:])
                    else:
                        nc.gpsimd.dma_start(y2_d[m0:m1, t0:t0 + BLK], yc[:])

            # blk0: A,C,B -- C bridges the w2-arrival wait, keeps HAM warm.
            # blk1: A, C-mh0 (bridges the u3-copy chain exactly like C0
            # does for B0), B, then C-mh1 so the tiny y2 write ends the
            # kernel.
            u3, xc8 = mm_a(0)
            mm_c(0)
            mm_b(0, u3, xc8)
            u3, xc8 = mm_a(1)
            mm_c(1, mh_list=[0])
            mm_b(1, u3, xc8, last=True)
            mm_c(1, mh_list=[1], out_sync=True)
    nc.finalize()
    return nc


_NC_CACHE = {}


def get_nc():
    if "nc" not in _NC_CACHE:
        _NC_CACHE["nc"] = _build_nc()
    return _NC_CACHE["nc"]


def _prep(A, B, sparse_weights1, sparse_weights2, weights_norms_rowwise,
          col_idx, col_comp_idx):
    bf16 = ml_dtypes.bfloat16
    f8 = ml_dtypes.float8_e4m3
    perm_in = np.concatenate([col_idx, col_comp_idx])
    w1t = (B * weights_norms_rowwise[None, :]).T.astype(np.float32)
    w1 = np.ascontiguousarray(
        w1t.reshape(2, 15, 128, RANK).transpose(0, 2, 1, 3)
    ).astype(bf16)
    w2 = np.ascontiguousarray(
        A.T.astype(np.float32).reshape(KT_B, 128, NKEEP).transpose(1, 0, 2)
    ).astype(bf16)
    s1 = np.ascontiguousarray(
        (sparse_weights1.T * S1S).astype(np.float32)
        .reshape(2, 128, NKEEP).transpose(1, 0, 2)
    ).astype(f8)
    s2t = (sparse_weights2[:, perm_in].T * S2S).astype(np.float32)
    s2 = np.ascontiguousarray(
        s2t.reshape(KT_ALL, 128, NCOMP).transpose(1, 0, 2)
    ).astype(f8)
    return w1, w2, s1, s2, perm_in


def kernel(x, A, B, sparse_weights1, sparse_weights2, weights_norms_rowwise,
           col_idx, col_comp_idx, row_idx, row_comp_idx):
    bf16 = ml_dtypes.bfloat16
    x = np.asarray(x, dtype=np.float32)
    w1, w2, s1, s2, perm_in = _prep(
        np.asarray(A, np.float32), np.asarray(B, np.float32),
        np.asarray(sparse_weights1, np.float32),
        np.asarray(sparse_weights2, np.float32),
        np.asarray(weights_norms_rowwise, np.float32),
        np.asarray(col_idx), np.asarray(col_comp_idx),
    )
    row_idx = np.asarray(row_idx)
    row_comp_idx = np.asarray(row_comp_idx)

    xs = x.reshape(TOK, N)
    in_maps = []
    for c in range(N_CORES):
        xcT = xs[c * TPC:(c + 1) * TPC][:, perm_in].T
        xb = np.ascontiguousarray(
            xcT.reshape(4, XCK, 128, NBLK, BLK).transpose(3, 0, 2, 1, 4)
        ).astype(bf16)
        in_maps.append({"x": xb, "w1": w1, "w2": w2, "s1": s1, "s2": s2})

    nc = get_nc()
    res = run_bass_kernel_spmd(nc, in_maps, core_ids=list(range(N_CORES)))
    globals()["_LAST_RESULTS"] = res
    y_rows = np.concatenate(
        [np.asarray(res.results[c]["y"], dtype=np.float32) for c in range(N_CORES)],
        axis=0,
    )
    y_comp = np.concatenate(
        [np.asarray(res.results[c]["y2"], dtype=np.float32) for c in range(N_CORES)],
        axis=1,
    )
    y = np.empty((TOK, N), dtype=np.float32)
    y[:, row_idx] = y_rows
    y[:, row_comp_idx] = y_comp.T
    return np.ascontiguousarray(y.reshape(x.shape))


# revision 46
# speedup vs baseline: 1.0064x; 1.0064x over previous
"""Low_Rank_linear Trainium2 kernel, v16 (152.7-154.3us across runs,
rel err 1.28e-2).

Per 512-token block (data-parallel over 8 cores, host-permuted inputs,
x pre-transposed feature-major bf16):
    MM-A  hidden.T = (B*wnorm) @ xp.T          k=3840, out 512  bf16
    MM-B  yp[:,:3840] = hid @ A.T + xc @ s1.T  k=768, out 3840  bf16+fp8DR
    MM-C  y2 = (s2p*64) @ xp.T (feature-major) k=4096, out 256  fp8DR

Measured layout (zero mid-kernel tensor gaps): warmups 7.5-14.8,
A0 -> 40.6, C0 -> 47.3 (bridges w2 arrival), B0 -> 80.2, A1 -> 106.3,
C1-mh0 -> 109.8 (bridges the u3-copy chain), B1 -> 142.5,
C1-mh1 -> 146.1, tail ~6.5.  Stream 131.3us vs 128.8 floor.

Design rules (each violated once and measured; do not regress):
  - 106 warm-up matmuls run gaplessly from ~7.5us until the first
    x/w1 chunks land (~14.5us).  HAM locks 8/8 at ~10.8us and sticks;
    idle before the first real matmul resets the ramp.  Start-late is
    cheap (1:1); start-early costs up to 4x in DMA-starve stalls
    (supply jitters +-1.5us), so NDUMMY is sized for the slow case.
  - first w1/x chunks are 2-kt tiles; w1/x pieces of the same kt
    range ride OPPOSITE hw queues so both queues advance the
    consumption frontier together.
  - all late loads (s2, s1, w2, blk1 x) ride the sync queue only: a
    gated DMA-issue instruction blocks its issuing ENGINE, and the
    scalar engine must stay free for the x8 casts -- otherwise the
    compile-time tensor-queue scheduler sees MM-B's DR matmuls
    blocked on xc8 and reorders the whole stream around them.
  - u3 psum->sbuf copies split across vector+scalar (serial 4-copy
    chain otherwise gates MM-B's first LDW); C1-mh0 emitted between
    A1 and B1 to cover what remains.
  - tail: B1's last yob half-row write split across scalar+sync, and
    the final y2 write is one mul + two half-writes on the HW queues
    (gpsimd's software queue needs ~1.3us for the same bytes).  Do
    NOT split C1-mh1's matmul chain or serialize two yc muls -- every
    such variant (v9, v13, v17) measured +0.5 to +1.3us, apparently
    from extra semaphores lengthening the fixed teardown chain.

Hard limits found (do not retry): gpsimd is a software DMA queue that
steals shared HBM bandwidth (~350-440GB/s total across all queues);
fp8 on the dominant MM-A/MM-B path exceeds the 2e-2 error budget
(each fp8 operand adds ~4%); the A@B' singular spectrum is flat, so
rank truncation / mixed-precision-by-singular-value loses; int8
matmul is unsupported by the toolchain; DoublePixel/DoubleColumn are
uint8-only; DR streams 1 col/cycle (2x via doubled k), so MM-B's
chunk cadence is already within 2.2% of floor.  Fixed costs: 7.5us
framework preamble, ~5us first-DMA latency, ~4us teardown."""

import numpy as np
import ml_dtypes

import concourse.bacc as bacc
import concourse.tile as tile
import concourse.mybir as mybir
from concourse.bass_utils import run_bass_kernel_spmd

N_CORES = 8
TOK = 8192
TPC = TOK // N_CORES  # 1024 tokens per core
N = 4096
RANK = 512
NKEEP = 3840
NCOMP = 256
BLK = 512             # token block (matmul moving N)
TT = 128              # token tile (stationary partition dim)
NBLK = TPC // BLK     # 2
KT_ALL = N // 128     # 32
KT_A = NKEEP // 128   # 30
KT_B = RANK // 128    # 4
NCH = 8
CW = NKEEP // NCH     # 480
XCK = 8               # k-tiles per full x chunk
S1S = 8.0
S2S = 64.0
NDUMMY = 97
HW = NKEEP // 2       # 1920, y half-row width

_BF16 = mybir.dt.bfloat16
_F32 = mybir.dt.float32
_F8 = mybir.dt.float8e4
_DR = mybir.MatmulPerfMode.DoubleRow


def _build_nc():
    nc = bacc.Bacc(None)
    x_d = nc.dram_tensor("x", [NBLK, 4, 128, XCK, BLK], _BF16, kind="ExternalInput")
    w1_d = nc.dram_tensor("w1", [2, 128, 15, RANK], _BF16, kind="ExternalInput")
    w2_d = nc.dram_tensor("w2", [128, KT_B, NKEEP], _BF16, kind="ExternalInput")
    s1_d = nc.dram_tensor("s1", [128, 2, NKEEP], _F8, kind="ExternalInput")
    s2_d = nc.dram_tensor("s2", [128, KT_ALL, NCOMP], _F8, kind="ExternalInput")
    y_d = nc.dram_tensor("y", [TPC, NKEEP], _BF16, kind="ExternalOutput")
    y2_d = nc.dram_tensor("y2", [NCOMP, TPC], _BF16, kind="ExternalOutput")

    with tile.TileContext(nc) as tc:
        with (
            tc.tile_pool(name="w1h2", bufs=2) as w1h2_pool,
            tc.tile_pool(name="w14", bufs=1) as w14_pool,
            tc.tile_pool(name="xh2", bufs=2) as xh2_pool,
            tc.tile_pool(name="w1", bufs=3) as w1_pool,
            tc.tile_pool(name="w2", bufs=4) as w2_pool,
            tc.tile_pool(name="s1", bufs=1) as s1_pool,
            tc.tile_pool(name="s2", bufs=1) as s2_pool,
            tc.tile_pool(name="xt4", bufs=1) as xt4_pool,
            tc.tile_pool(name="xt", bufs=7) as xt_pool,
            tc.tile_pool(name="x8", bufs=1) as x8_pool,
            tc.tile_pool(name="xc8", bufs=2) as xc8_pool,
            tc.tile_pool(name="u3", bufs=2) as u3_pool,
            tc.tile_pool(name="yoa", bufs=2) as yoa_pool,
            tc.tile_pool(name="yob", bufs=2) as yob_pool,
            tc.tile_pool(name="yc", bufs=2) as yc_pool,
            tc.tile_pool(name="wrm", bufs=1) as wrm_pool,
            tc.tile_pool(name="psA", bufs=4, space="PSUM") as psA,
            tc.tile_pool(name="psB", bufs=2, space="PSUM") as psB,
            tc.tile_pool(name="psC", bufs=2, space="PSUM") as psC,
        ):
            # --- tiles ---------------------------------------------------
            # blk0 chunk layout: 2,2,4 k-tiles then three 8kt chunks
            w1h2 = [w1h2_pool.tile([128, 2, RANK], _BF16, name="w1h2t")
                    for _ in range(2)]
            w1h = w14_pool.tile([128, 4, RANK], _BF16, name="w14t")
            w1f = [w1_pool.tile([128, XCK, RANK], _BF16, name="w1sb")
                   for _ in range(3)]
            # (tile, kt_start, nkt) per A-chunk
            w1_chunks = [(w1h2[0], 0, 2), (w1h2[1], 2, 2), (w1h, 4, 4),
                         (w1f[0], 8, 8), (w1f[1], 16, 8), (w1f[2], 24, 6)]
            w2_sb = [w2_pool.tile([128, KT_B, 2 * CW], _BF16, name="w2sb")
                     for c in range(4)]
            s1_sb = s1_pool.tile([128, 2, NKEEP], _F8)
            s2_sb = s2_pool.tile([128, KT_ALL, NCOMP], _F8)
            xh2 = [xh2_pool.tile([128, 2, BLK], _BF16, name="xh2t")
                   for _ in range(2)]
            xt0h = xt4_pool.tile([128, 4, BLK], _BF16, name="xt4t")
            xt_f = [xt_pool.tile([128, XCK, BLK], _BF16, name="xts")
                    for _ in range(7)]
            # per-block x chunk lists: (tile, kt_start, nkt)
            x_chunks = [
                [(xh2[0], 0, 2), (xh2[1], 2, 2), (xt0h, 4, 4),
                 (xt_f[0], 8, 8), (xt_f[1], 16, 8), (xt_f[2], 24, 8)],
                [(xt_f[3], 0, 8), (xt_f[4], 8, 8), (xt_f[5], 16, 8),
                 (xt_f[6], 24, 8)],
            ]
            x8_sb = x8_pool.tile([128, KT_ALL, BLK], _F8)

            # --- warm-up (HAM 8/8 before real MMs) -----------------------
            wrm = wrm_pool.tile([128, 128], _BF16)
            wps = psA.tile([128, 128], _F32, name="psa")
            nc.gpsimd.memset(wrm[:], 0.0)
            for i in range(NDUMMY):
                nc.tensor.matmul(wps[:], wrm[:], wrm[:], start=True, stop=True)

            # --- need-ordered loads (both HW queues ~220GB/s; gpsimd's
            # software queue steals shared bandwidth, so no loads there) --
            # w1/x pieces of the same kt range ride OPPOSITE queues so the
            # two HW queues advance the consumption frontier together
            nc.sync.dma_start(w1h2[0][:], w1_d[0, :, :2])
            nc.scalar.dma_start(xh2[0][:], x_d[0, 0, :, :2, :])
            nc.scalar.dma_start(w1h2[1][:], w1_d[0, :, 2:4])
            nc.sync.dma_start(xh2[1][:], x_d[0, 0, :, 2:4, :])
            nc.sync.dma_start(w1h[:], w1_d[0, :, 4:8])
            nc.scalar.dma_start(xt0h[:], x_d[0, 0, :, 4:, :])
            nc.scalar.dma_start(w1f[0][:, :7, :], w1_d[0, :, 8:])
            nc.sync.dma_start(xt_f[0][:, :4, :], x_d[0, 1, :, :4, :])
            nc.scalar.dma_start(xt_f[0][:, 4:, :], x_d[0, 1, :, 4:, :])
            nc.sync.dma_start(w1f[0][:, 7:8, :], w1_d[1, :, :1])
            nc.sync.dma_start(w1f[1][:], w1_d[1, :, 1:9])
            nc.scalar.dma_start(xt_f[1][:], x_d[0, 2])
            nc.scalar.dma_start(w1f[2][:, :6, :], w1_d[1, :, 9:])
            nc.sync.dma_start(xt_f[2][:], x_d[0, 3])
            # all late loads ride sync: gated DMA-issue instructions block
            # the issuing ENGINE, and scalar must stay free for the x8
            # casts (else the compile-time scheduler defers MM-B's DR
            # matmuls and scrambles the whole tensor queue)
            nc.sync.dma_start(s2_sb[:], s2_d[:])
            nc.sync.dma_start(s1_sb[:], s1_d[:])
            nc.sync.dma_start(w2_sb[0][:], w2_d[:, :, 0:2 * CW])
            nc.sync.dma_start(w2_sb[1][:], w2_d[:, :, 2 * CW:4 * CW])
            nc.sync.dma_start(w2_sb[2][:], w2_d[:, :, 4 * CW:6 * CW])
            nc.sync.dma_start(w2_sb[3][:], w2_d[:, :, 6 * CW:])
            nc.sync.dma_start(xt_f[3][:], x_d[1, 0])
            nc.sync.dma_start(xt_f[4][:], x_d[1, 1])
            nc.sync.dma_start(xt_f[5][:], x_d[1, 2])
            nc.sync.dma_start(xt_f[6][:], x_d[1, 3])

            # --- per-block compute ---------------------------------------
            def mm_a(blk):
                u3 = u3_pool.tile([128, KT_B, BLK], _BF16)
                psa = [psA.tile([128, BLK], _F32, name="psa")
                       for m in range(RANK // 128)]
                for (xt, kt0, nkt) in x_chunks[blk]:
                    for j in range(nkt):
                        kt = kt0 + j
                        if kt >= KT_A:
                            continue
                        ci = (0 if kt < 2 else 1 if kt < 4 else 2 if kt < 8
                              else 3 + (kt - 8) // 8)
                        wt, wkt0, _ = w1_chunks[ci]
                        for m in range(RANK // 128):
                            nc.tensor.matmul(
                                psa[m][:],
                                wt[:, kt - wkt0, m * 128:(m + 1) * 128],
                                xt[:, j, :],
                                start=(kt == 0),
                                stop=(kt == KT_A - 1),
                            )
                    # fp8 cast for MM-C, hidden in MM-A's DMA-paced window
                    nc.scalar.copy(out=x8_sb[:, kt0:kt0 + nkt, :], in_=xt[:])
                # u3 copies split across vector+scalar: the serial 4-copy
                # chain (~2.6us on vector alone) gates MM-B's first LDW
                nc.vector.tensor_copy(out=u3[:, 0, :], in_=psa[0][:])
                nc.scalar.copy(out=u3[:, 2, :], in_=psa[2][:])
                nc.vector.tensor_copy(out=u3[:, 1, :], in_=psa[1][:])
                nc.scalar.copy(out=u3[:, 3, :], in_=psa[3][:])
                xc8 = xc8_pool.tile([128, 2, BLK], _F8)
                nc.scalar.mul(xc8[:], x8_sb[:, 30:32, :], 1.0 / S1S)
                return u3, xc8

            def mm_b(blk, u3, xc8, last=False):
                t0 = blk * BLK
                for mt in range(BLK // TT):
                    yoa = yoa_pool.tile([128, HW], _BF16)
                    yob = yob_pool.tile([128, HW], _BF16)
                    for n in range(NCH):
                        ps = psB.tile([128, CW], _F32)
                        for kt in range(KT_B):
                            nc.tensor.matmul(
                                ps[:],
                                u3[:, kt, mt * TT:(mt + 1) * TT],
                                w2_sb[n // 2][:, kt,
                                              (n % 2) * CW:(n % 2 + 1) * CW],
                                start=(kt == 0),
                                stop=False,
                            )
                        nc.tensor.matmul(
                            ps[:],
                            xc8[:, :, mt * TT:(mt + 1) * TT],
                            s1_sb[:, :, n * CW:(n + 1) * CW],
                            start=False,
                            stop=True,
                            perf_mode=_DR,
                        )
                        dst = yoa if n < 4 else yob
                        nc.vector.tensor_copy(
                            out=dst[:, (n % 4) * CW:(n % 4 + 1) * CW], in_=ps[:]
                        )
                        if n == 3:
                            nc.scalar.dma_start(
                                y_d[t0 + mt * TT:t0 + (mt + 1) * TT, :HW],
                                yoa[:],
                            )
                    r0, r1 = t0 + mt * TT, t0 + (mt + 1) * TT
                    if last and mt == BLK // TT - 1:
                        # the kernel's final row writes gate teardown:
                        # split across both HW queues so they land ~1us
                        # earlier than one serial 480KB transfer
                        nc.scalar.dma_start(y_d[r0:r1, HW:HW + 960],
                                            yob[:, :960])
                        nc.sync.dma_start(y_d[r0:r1, HW + 960:],
                                          yob[:, 960:])
                    else:
                        nc.scalar.dma_start(y_d[r0:r1, HW:], yob[:])

            def mm_c(blk, mh_list=None, out_sync=False):
                t0 = blk * BLK
                for mh in (mh_list or range(NCOMP // 128)):
                    m0, m1 = mh * 128, (mh + 1) * 128
                    ps = psC.tile([128, BLK], _F32)
                    for k2 in range(KT_ALL // 2):
                        nc.tensor.matmul(
                            ps[:],
                            s2_sb[:, 2 * k2:2 * k2 + 2, m0:m1],
                            x8_sb[:, 2 * k2:2 * k2 + 2, :],
                            start=(k2 == 0),
                            stop=(k2 == KT_ALL // 2 - 1),
                            perf_mode=_DR,
                        )
                    yc = yc_pool.tile([128, BLK], _BF16)
                    nc.scalar.mul(yc[:], ps[:], 1.0 / S2S)
                    if out_sync:
                        # final y2 write gates teardown: one mul, then the
                        # write split across both idle HW queues (gpsimd's
                        # software queue needs ~1.3us for the same bytes)
                        nc.sync.dma_start(y2_d[m0:m1, t0:t0 + 256],
                                          yc[:, :256])
                        nc.scalar.dma_start(y2_d[m0:m1, t0 + 256:t0 + BLK],
                                            yc[:, 256:])
                    else:
                        nc.gpsimd.dma_start(y2_d[m0:m1, t0:t0 + BLK], yc[:])

            # blk0: A,C,B -- C bridges the w2-arrival wait, keeps HAM warm.
            # blk1: A, C-mh0 (bridges the u3-copy chain exactly like C0
            # does for B0), B, then C-mh1 so the tiny y2 write ends the
            # kernel.
            u3, xc8 = mm_a(0)
            mm_c(0)
            mm_b(0, u3, xc8)
            u3, xc8 = mm_a(1)
            mm_c(1, mh_list=[0])
            mm_b(1, u3, xc8, last=True)
            mm_c(1, mh_list=[1], out_sync=True)
    nc.finalize()
    return nc


_NC_CACHE = {}


def get_nc():
    if "nc" not in _NC_CACHE:
        _NC_CACHE["nc"] = _build_nc()
    return _NC_CACHE["nc"]


def _prep(A, B, sparse_weights1, sparse_weights2, weights_norms_rowwise,
          col_idx, col_comp_idx):
    bf16 = ml_dtypes.bfloat16
    f8 = ml_dtypes.float8_e4m3
    perm_in = np.concatenate([col_idx, col_comp_idx])
    w1t = (B * weights_norms_rowwise[None, :]).T.astype(np.float32)
    w1 = np.ascontiguousarray(
        w1t.reshape(2, 15, 128, RANK).transpose(0, 2, 1, 3)
    ).astype(bf16)
    w2 = np.ascontiguousarray(
        A.T.astype(np.float32).reshape(KT_B, 128, NKEEP).transpose(1, 0, 2)
    ).astype(bf16)
    s1 = np.ascontiguousarray(
        (sparse_weights1.T * S1S).astype(np.float32)
        .reshape(2, 128, NKEEP).transpose(1, 0, 2)
    ).astype(f8)
    s2t = (sparse_weights2[:, perm_in].T * S2S).astype(np.float32)
    s2 = np.ascontiguousarray(
        s2t.reshape(KT_ALL, 128, NCOMP).transpose(1, 0, 2)
    ).astype(f8)
    return w1, w2, s1, s2, perm_in


def kernel(x, A, B, sparse_weights1, sparse_weights2, weights_norms_rowwise,
           col_idx, col_comp_idx, row_idx, row_comp_idx):
    bf16 = ml_dtypes.bfloat16
    x = np.asarray(x, dtype=np.float32)
    w1, w2, s1, s2, perm_in = _prep(
        np.asarray(A, np.float32), np.asarray(B, np.float32),
        np.asarray(sparse_weights1, np.float32),
        np.asarray(sparse_weights2, np.float32),
        np.asarray(weights_norms_rowwise, np.float32),
        np.asarray(col_idx), np.asarray(col_comp_idx),
    )
    row_idx = np.asarray(row_idx)
    row_comp_idx = np.asarray(row_comp_idx)

    xs = x.reshape(TOK, N)
    in_maps = []
    for c in range(N_CORES):
        xcT = xs[c * TPC:(c + 1) * TPC][:, perm_in].T
        xb = np.ascontiguousarray(
            xcT.reshape(4, XCK, 128, NBLK, BLK).transpose(3, 0, 2, 1, 4)
        ).astype(bf16)
        in_maps.append({"x": xb, "w1": w1, "w2": w2, "s1": s1, "s2": s2})

    nc = get_nc()
    res = run_bass_kernel_spmd(nc, in_maps, core_ids=list(range(N_CORES)))
    globals()["_LAST_RESULTS"] = res
    y_rows = np.concatenate(
        [np.asarray(res.results[c]["y"], dtype=np.float32) for c in range(N_CORES)],
        axis=0,
    )
    y_comp = np.concatenate(
        [np.asarray(res.results[c]["y2"], dtype=np.float32) for c in range(N_CORES)],
        axis=1,
    )
    y = np.empty((TOK, N), dtype=np.float32)
    y[:, row_idx] = y_rows
    y[:, row_comp_idx] = y_comp.T
    return np.ascontiguousarray(y.reshape(x.shape))


# revision 55
# speedup vs baseline: 1.0200x; 1.0135x over previous
"""Low_Rank_linear Trainium2 kernel, v16 (152.7-154.3us across runs,
rel err 1.28e-2).

Per 512-token block (data-parallel over 8 cores, host-permuted inputs,
x pre-transposed feature-major bf16):
    MM-A  hidden.T = (B*wnorm) @ xp.T          k=3840, out 512  bf16
    MM-B  yp[:,:3840] = hid @ A.T + xc @ s1.T  k=768, out 3840  bf16+fp8DR
    MM-C  y2 = (s2p*64) @ xp.T (feature-major) k=4096, out 256  fp8DR

Measured layout (zero mid-kernel tensor gaps): warmups 7.5-14.8,
A0 -> 40.6, C0 -> 47.3 (bridges w2 arrival), B0 -> 80.2, A1 -> 106.3,
C1-mh0 -> 109.8 (bridges the u3-copy chain), B1 -> 142.5,
C1-mh1 -> 146.1, tail ~6.5.  Stream 131.3us vs 128.8 floor.

Design rules (each violated once and measured; do not regress):
  - 106 warm-up matmuls run gaplessly from ~7.5us until the first
    x/w1 chunks land (~14.5us).  HAM locks 8/8 at ~10.8us and sticks;
    idle before the first real matmul resets the ramp.  Start-late is
    cheap (1:1); start-early costs up to 4x in DMA-starve stalls
    (supply jitters +-1.5us), so NDUMMY is sized for the slow case.
  - first w1/x chunks are 2-kt tiles; w1/x pieces of the same kt
    range ride OPPOSITE hw queues so both queues advance the
    consumption frontier together.
  - all late loads (s2, s1, w2, blk1 x) ride the sync queue only: a
    gated DMA-issue instruction blocks its issuing ENGINE, and the
    scalar engine must stay free for the x8 casts -- otherwise the
    compile-time tensor-queue scheduler sees MM-B's DR matmuls
    blocked on xc8 and reorders the whole stream around them.
  - u3 psum->sbuf copies split across vector+scalar (serial 4-copy
    chain otherwise gates MM-B's first LDW); C1-mh0 emitted between
    A1 and B1 to cover what remains.
  - tail: B1's last yob half-row write split across scalar+sync, and
    the final y2 write is one mul + two half-writes on the HW queues
    (gpsimd's software queue needs ~1.3us for the same bytes).  Do
    NOT split C1-mh1's matmul chain or serialize two yc muls -- every
    such variant (v9, v13, v17) measured +0.5 to +1.3us, apparently
    from extra semaphores lengthening the fixed teardown chain.

Hard limits found (do not retry): gpsimd is a software DMA queue that
steals shared HBM bandwidth (~350-440GB/s total across all queues);
fp8 on the dominant MM-A/MM-B path exceeds the 2e-2 error budget
(each fp8 operand adds ~4%); the A@B' singular spectrum is flat, so
rank truncation / mixed-precision-by-singular-value loses; int8
matmul is unsupported by the toolchain; DoublePixel/DoubleColumn are
uint8-only; DR streams 1 col/cycle (2x via doubled k), so MM-B's
chunk cadence is already within 2.2% of floor.  Fixed costs: 7.5us
framework preamble, ~5us first-DMA latency, ~4us teardown."""

import contextlib

import numpy as np
import ml_dtypes

import concourse.bacc as bacc
import concourse.tile as tile
import concourse.mybir as mybir
from concourse.bass_utils import run_bass_kernel_spmd

N_CORES = 8
TOK = 8192
TPC = TOK // N_CORES  # 1024 tokens per core
N = 4096
RANK = 512
NKEEP = 3840
NCOMP = 256
BLK = 512             # token block (matmul moving N)
TT = 128              # token tile (stationary partition dim)
NBLK = TPC // BLK     # 2
KT_ALL = N // 128     # 32
KT_A = NKEEP // 128   # 30
KT_B = RANK // 128    # 4
NCH = 8
CW = NKEEP // NCH     # 480
XCK = 8               # k-tiles per full x chunk
S1S = 8.0
S2S = 64.0
NDUMMY = 97
HW = NKEEP // 2       # 1920, y half-row width

_BF16 = mybir.dt.bfloat16
_F32 = mybir.dt.float32
_F8 = mybir.dt.float8e4
_DR = mybir.MatmulPerfMode.DoubleRow


def _build_nc():
    nc = bacc.Bacc(None)
    x_d = nc.dram_tensor("x", [NBLK, 4, 128, XCK, BLK], _BF16, kind="ExternalInput")
    w1_d = nc.dram_tensor("w1", [2, 128, 15, RANK], _BF16, kind="ExternalInput")
    w2_d = nc.dram_tensor("w2", [128, KT_B, NKEEP], _BF16, kind="ExternalInput")
    s1_d = nc.dram_tensor("s1", [128, 2, NKEEP], _F8, kind="ExternalInput")
    s2_d = nc.dram_tensor("s2", [128, KT_ALL, NCOMP], _F8, kind="ExternalInput")
    y_d = nc.dram_tensor("y", [TPC, NKEEP], _BF16, kind="ExternalOutput")
    y2_d = nc.dram_tensor("y2", [NCOMP, TPC], _BF16, kind="ExternalOutput")

    with tile.TileContext(nc) as tc, contextlib.ExitStack() as es:
        if True:
            p = lambda **kw: es.enter_context(tc.tile_pool(**kw))
            w1h2_pool = p(name="w1h2", bufs=2)
            w14_pool = p(name="w14", bufs=1)
            xh2_pool = p(name="xh2", bufs=2)
            w1q_pool = p(name="w1q", bufs=5)
            w1t_pool = p(name="w1t", bufs=1)
            xq_pool = p(name="xq", bufs=6)
            w2_pool = p(name="w2", bufs=4)
            s1_pool = p(name="s1", bufs=1)
            s2_pool = p(name="s2", bufs=1)
            xt4_pool = p(name="xt4", bufs=1)
            xt_pool = p(name="xt", bufs=4)
            x8_pool = p(name="x8", bufs=1)
            xc8_pool = p(name="xc8", bufs=2)
            u3_pool = p(name="u3", bufs=2)
            yoa_pool = p(name="yoa", bufs=2)
            yob_pool = p(name="yob", bufs=2)
            yc_pool = p(name="yc", bufs=2)
            wrm_pool = p(name="wrm", bufs=1)
            psA = p(name="psA", bufs=4, space="PSUM")
            psB = p(name="psB", bufs=2, space="PSUM")
            psC = p(name="psC", bufs=2, space="PSUM")
            # --- tiles ---------------------------------------------------
            # blk0 chunk layout: 2,2,4 k-tiles then 4-kt tiles through
            # kt27 and a final 2-kt tile -- separate tiles give SUB-chunk
            # arrival granularity (whole-tile deps otherwise make
            # consumers wait for a full 1MB lump; the recurring ~29us
            # stall was the kt16-24 boundary)
            w1h2 = [w1h2_pool.tile([128, 2, RANK], _BF16, name="w1h2t")
                    for _ in range(2)]
            w1h = w14_pool.tile([128, 4, RANK], _BF16, name="w14t")
            w1q = [w1q_pool.tile([128, 4, RANK], _BF16, name="w1qt")
                   for _ in range(5)]
            w1t = w1t_pool.tile([128, 2, RANK], _BF16)
            # (tile, kt_start, nkt) per A-chunk
            w1_chunks = ([(w1h2[0], 0, 2), (w1h2[1], 2, 2), (w1h, 4, 4)]
                         + [(w1q[i], 8 + 4 * i, 4) for i in range(5)]
                         + [(w1t, 28, 2)])
            w2_sb = [w2_pool.tile([128, KT_B, 2 * CW], _BF16, name="w2sb")
                     for c in range(4)]
            s1_sb = s1_pool.tile([128, 2, NKEEP], _F8)
            s2_sb = s2_pool.tile([128, KT_ALL, NCOMP], _F8)
            xh2 = [xh2_pool.tile([128, 2, BLK], _BF16, name="xh2t")
                   for _ in range(2)]
            xt0h = xt4_pool.tile([128, 4, BLK], _BF16, name="xt4t")
            xq = [xq_pool.tile([128, 4, BLK], _BF16, name="xqt")
                  for _ in range(6)]
            xt_f = [xt_pool.tile([128, XCK, BLK], _BF16, name="xts")
                    for _ in range(4)]
            # per-block x chunk lists: (tile, kt_start, nkt)
            x_chunks = [
                [(xh2[0], 0, 2), (xh2[1], 2, 2), (xt0h, 4, 4)]
                + [(xq[i], 8 + 4 * i, 4) for i in range(6)],
                [(xt_f[i], 8 * i, 8) for i in range(4)],
            ]
            x8_sb = x8_pool.tile([128, KT_ALL, BLK], _F8)

            # --- warm-up (HAM 8/8 before real MMs) -----------------------
            wrm = wrm_pool.tile([128, 128], _BF16)
            wps = psA.tile([128, 128], _F32, name="psa")
            nc.gpsimd.memset(wrm[:], 0.0)
            for i in range(NDUMMY):
                nc.tensor.matmul(wps[:], wrm[:], wrm[:], start=True, stop=True)

            # --- need-ordered loads (both HW queues ~220GB/s; gpsimd's
            # software queue steals shared bandwidth, so no loads there) --
            # w1/x pieces of the same kt range ride OPPOSITE queues so the
            # two HW queues advance the consumption frontier together
            nc.sync.dma_start(w1h2[0][:], w1_d[0, :, :2])
            nc.scalar.dma_start(xh2[0][:], x_d[0, 0, :, :2, :])
            nc.scalar.dma_start(w1h2[1][:], w1_d[0, :, 2:4])
            nc.sync.dma_start(xh2[1][:], x_d[0, 0, :, 2:4, :])
            nc.sync.dma_start(w1h[:], w1_d[0, :, 4:8])
            nc.scalar.dma_start(xt0h[:], x_d[0, 0, :, 4:, :])
            nc.scalar.dma_start(w1q[0][:], w1_d[0, :, 8:12])
            nc.sync.dma_start(xq[0][:], x_d[0, 1, :, :4, :])
            nc.sync.dma_start(w1q[1][:, :3, :], w1_d[0, :, 12:15])
            nc.sync.dma_start(w1q[1][:, 3:4, :], w1_d[1, :, :1])
            nc.scalar.dma_start(xq[1][:], x_d[0, 1, :, 4:, :])
            nc.scalar.dma_start(w1q[2][:], w1_d[1, :, 1:5])
            nc.sync.dma_start(xq[2][:], x_d[0, 2, :, :4, :])
            nc.sync.dma_start(w1q[3][:], w1_d[1, :, 5:9])
            nc.scalar.dma_start(xq[3][:], x_d[0, 2, :, 4:, :])
            nc.scalar.dma_start(w1q[4][:], w1_d[1, :, 9:13])
            nc.sync.dma_start(xq[4][:], x_d[0, 3, :, :4, :])
            nc.sync.dma_start(w1t[:], w1_d[1, :, 13:15])
            nc.scalar.dma_start(xq[5][:], x_d[0, 3, :, 4:, :])
            # all late loads ride sync: gated DMA-issue instructions block
            # the issuing ENGINE, and scalar must stay free for the x8
            # casts (else the compile-time scheduler defers MM-B's DR
            # matmuls and scrambles the whole tensor queue)
            nc.sync.dma_start(s2_sb[:], s2_d[:])
            nc.sync.dma_start(s1_sb[:], s1_d[:])
            nc.sync.dma_start(w2_sb[0][:], w2_d[:, :, 0:2 * CW])
            nc.sync.dma_start(w2_sb[1][:], w2_d[:, :, 2 * CW:4 * CW])
            nc.sync.dma_start(w2_sb[2][:], w2_d[:, :, 4 * CW:6 * CW])
            nc.sync.dma_start(w2_sb[3][:], w2_d[:, :, 6 * CW:])
            nc.sync.dma_start(xt_f[0][:], x_d[1, 0])
            nc.sync.dma_start(xt_f[1][:], x_d[1, 1])
            nc.sync.dma_start(xt_f[2][:], x_d[1, 2])
            nc.sync.dma_start(xt_f[3][:], x_d[1, 3])

            # --- per-block compute ---------------------------------------
            def mm_a(blk):
                u3 = u3_pool.tile([128, KT_B, BLK], _BF16)
                psa = [psA.tile([128, BLK], _F32, name="psa")
                       for m in range(RANK // 128)]
                for (xt, kt0, nkt) in x_chunks[blk]:
                    for j in range(nkt):
                        kt = kt0 + j
                        if kt >= KT_A:
                            continue
                        ci = (0 if kt < 2 else 1 if kt < 4 else 2 if kt < 8
                              else 3 + (kt - 8) // 4)
                        wt, wkt0, _ = w1_chunks[ci]
                        for m in range(RANK // 128):
                            nc.tensor.matmul(
                                psa[m][:],
                                wt[:, kt - wkt0, m * 128:(m + 1) * 128],
                                xt[:, j, :],
                                start=(kt == 0),
                                stop=(kt == KT_A - 1),
                            )
                    # fp8 cast for MM-C, hidden in MM-A's DMA-paced window
                    nc.scalar.copy(out=x8_sb[:, kt0:kt0 + nkt, :], in_=xt[:])
                # u3 copies split across vector+scalar: the serial 4-copy
                # chain (~2.6us on vector alone) gates MM-B's first LDW
                nc.vector.tensor_copy(out=u3[:, 0, :], in_=psa[0][:])
                nc.scalar.copy(out=u3[:, 2, :], in_=psa[2][:])
                nc.vector.tensor_copy(out=u3[:, 1, :], in_=psa[1][:])
                nc.scalar.copy(out=u3[:, 3, :], in_=psa[3][:])
                xc8 = xc8_pool.tile([128, 2, BLK], _F8)
                nc.scalar.mul(xc8[:], x8_sb[:, 30:32, :], 1.0 / S1S)
                return u3, xc8

            def mm_b(blk, u3, xc8, last=False):
                t0 = blk * BLK
                for mt in range(BLK // TT):
                    yoa = yoa_pool.tile([128, HW], _BF16)
                    yob = yob_pool.tile([128, HW], _BF16)
                    for n in range(NCH):
                        ps = psB.tile([128, CW], _F32)
                        for kt in range(KT_B):
                            nc.tensor.matmul(
                                ps[:],
                                u3[:, kt, mt * TT:(mt + 1) * TT],
                                w2_sb[n // 2][:, kt,
                                              (n % 2) * CW:(n % 2 + 1) * CW],
                                start=(kt == 0),
                                stop=False,
                            )
                        nc.tensor.matmul(
                            ps[:],
                            xc8[:, :, mt * TT:(mt + 1) * TT],
                            s1_sb[:, :, n * CW:(n + 1) * CW],
                            start=False,
                            stop=True,
                            perf_mode=_DR,
                        )
                        dst = yoa if n < 4 else yob
                        nc.vector.tensor_copy(
                            out=dst[:, (n % 4) * CW:(n % 4 + 1) * CW], in_=ps[:]
                        )
                        if n == 3:
                            nc.scalar.dma_start(
                                y_d[t0 + mt * TT:t0 + (mt + 1) * TT, :HW],
                                yoa[:],
                            )
                    r0, r1 = t0 + mt * TT, t0 + (mt + 1) * TT
                    if last and mt == BLK // TT - 1:
                        # the kernel's final row writes gate teardown:
                        # split across both HW queues so they land ~1us
                        # earlier than one serial 480KB transfer
                        nc.scalar.dma_start(y_d[r0:r1, HW:HW + 960],
                                            yob[:, :960])
                        nc.sync.dma_start(y_d[r0:r1, HW + 960:],
                                          yob[:, 960:])
                    else:
                        nc.scalar.dma_start(y_d[r0:r1, HW:], yob[:])

            def mm_c(blk, mh_list=None, out_sync=False):
                t0 = blk * BLK
                for mh in (mh_list or range(NCOMP // 128)):
                    m0, m1 = mh * 128, (mh + 1) * 128
                    ps = psC.tile([128, BLK], _F32)
                    for k2 in range(KT_ALL // 2):
                        nc.tensor.matmul(
                            ps[:],
                            s2_sb[:, 2 * k2:2 * k2 + 2, m0:m1],
                            x8_sb[:, 2 * k2:2 * k2 + 2, :],
                            start=(k2 == 0),
                            stop=(k2 == KT_ALL // 2 - 1),
                            perf_mode=_DR,
                        )
                    yc = yc_pool.tile([128, BLK], _BF16)
                    nc.scalar.mul(yc[:], ps[:], 1.0 / S2S)
                    if out_sync:
                        # final y2 write gates teardown: one mul, then the
                        # write split across both idle HW queues (gpsimd's
                        # software queue needs ~1.3us for the same bytes)
                        nc.sync.dma_start(y2_d[m0:m1, t0:t0 + 256],
                                          yc[:, :256])
                        nc.scalar.dma_start(y2_d[m0:m1, t0 + 256:t0 + BLK],
                                            yc[:, 256:])
                    else:
                        nc.gpsimd.dma_start(y2_d[m0:m1, t0:t0 + BLK], yc[:])

            # blk0: A,C,B -- C bridges the w2-arrival wait, keeps HAM warm.
            # blk1: A, C-mh0 (bridges the u3-copy chain exactly like C0
            # does for B0), B, then C-mh1 so the tiny y2 write ends the
            # kernel.
            u3, xc8 = mm_a(0)
            mm_c(0)
            mm_b(0, u3, xc8)
            u3, xc8 = mm_a(1)
            mm_c(1, mh_list=[0])
            mm_b(1, u3, xc8, last=True)
            mm_c(1, mh_list=[1], out_sync=True)
    nc.finalize()
    return nc


_NC_CACHE = {}


def get_nc():
    if "nc" not in _NC_CACHE:
        _NC_CACHE["nc"] = _build_nc()
    return _NC_CACHE["nc"]


def _prep(A, B, sparse_weights1, sparse_weights2, weights_norms_rowwise,
          col_idx, col_comp_idx):
    bf16 = ml_dtypes.bfloat16
    f8 = ml_dtypes.float8_e4m3
    perm_in = np.concatenate([col_idx, col_comp_idx])
    w1t = (B * weights_norms_rowwise[None, :]).T.astype(np.float32)
    w1 = np.ascontiguousarray(
        w1t.reshape(2, 15, 128, RANK).transpose(0, 2, 1, 3)
    ).astype(bf16)
    w2 = np.ascontiguousarray(
        A.T.astype(np.float32).reshape(KT_B, 128, NKEEP).transpose(1, 0, 2)
    ).astype(bf16)
    s1 = np.ascontiguousarray(
        (sparse_weights1.T * S1S).astype(np.float32)
        .reshape(2, 128, NKEEP).transpose(1, 0, 2)
    ).astype(f8)
    s2t = (sparse_weights2[:, perm_in].T * S2S).astype(np.float32)
    s2 = np.ascontiguousarray(
        s2t.reshape(KT_ALL, 128, NCOMP).transpose(1, 0, 2)
    ).astype(f8)
    return w1, w2, s1, s2, perm_in


def kernel(x, A, B, sparse_weights1, sparse_weights2, weights_norms_rowwise,
           col_idx, col_comp_idx, row_idx, row_comp_idx):
    bf16 = ml_dtypes.bfloat16
    x = np.asarray(x, dtype=np.float32)
    w1, w2, s1, s2, perm_in = _prep(
        np.asarray(A, np.float32), np.asarray(B, np.float32),
        np.asarray(sparse_weights1, np.float32),
        np.asarray(sparse_weights2, np.float32),
        np.asarray(weights_norms_rowwise, np.float32),
        np.asarray(col_idx), np.asarray(col_comp_idx),
    )
    row_idx = np.asarray(row_idx)
    row_comp_idx = np.asarray(row_comp_idx)

    xs = x.reshape(TOK, N)
    in_maps = []
    for c in range(N_CORES):
        xcT = xs[c * TPC:(c + 1) * TPC][:, perm_in].T
        xb = np.ascontiguousarray(
            xcT.reshape(4, XCK, 128, NBLK, BLK).transpose(3, 0, 2, 1, 4)
        ).astype(bf16)
        in_maps.append({"x": xb, "w1": w1, "w2": w2, "s1": s1, "s2": s2})

    nc = get_nc()
    res = run_bass_kernel_spmd(nc, in_maps, core_ids=list(range(N_CORES)))
    globals()["_LAST_RESULTS"] = res
    y_rows = np.concatenate(
        [np.asarray(res.results[c]["y"], dtype=np.float32) for c in range(N_CORES)],
        axis=0,
    )
    y_comp = np.concatenate(
        [np.asarray(res.results[c]["y2"], dtype=np.float32) for c in range(N_CORES)],
        axis=1,
    )
    y = np.empty((TOK, N), dtype=np.float32)
    y[:, row_idx] = y_rows
    y[:, row_comp_idx] = y_comp.T
    return np.ascontiguousarray(y.reshape(x.shape))
